# revision 43
# baseline (speedup 1.0000x reference)
"""Cosine attention (nn_CosineAttention) Trainium2 Bass kernel.

Sharding: 8 cores = 4 batches x 2 head-groups (4 heads each).
Each core: LN(q/k/v) -> project with gamma/beta folded into weights ->
cosine attention for its 4 heads -> partial output projection with its
W_out row-slice. Host sums the two head-group partials per batch and
adds b_out.

Math notes:
 - LN(x) @ Wq == ((x-mu)*rstd) @ (gamma[:,None]*Wq) + beta@Wq : exact fold.
 - attn = (qh.kh)/(|qh||kh|+eps): eps folded (rel err ~1e-10); q-side norm
   applied to q-hat before scores, k-side norm applied on the score
   psum->sbuf copy via a broadcast-view multiply (score_T rows are k rows).

Schedule: emission order software-pipelines the phases so the PE stays fed:
k-LN -> q-LN -> k-proj -> k-norms -> q-proj -> q-norms/qhat -> v-LN ->
scores(0) -> v-proj -> scores/accum pipeline -> final projection.
Score PSUM tiles pair two k-tiles (2 banks) so one DVE op scales+copies
1024 columns; timing reps run the body in a hardware For_i loop so the
NEFF stays constant across rep counts.
"""

import sys

sys.path.insert(0, "/opt/trn_rl_repo")

import numpy as np
import ml_dtypes

import concourse.bass as bass  # noqa: F401  (AP helpers)
import concourse.mybir as mybir
import concourse.tile as tile
from concourse import bacc, bass_utils

BF16 = mybir.dt.bfloat16
F32 = mybir.dt.float32
AF = mybir.ActivationFunctionType
MUL = mybir.AluOpType.mult
ADD = mybir.AluOpType.add

HEADS = 8
DH = 64
DIM = 512
NQ = 1024
NK = 2048
B = 4
N_CORES = 8
HG = 2            # head groups (cores per batch)
LH = HEADS // HG  # local heads per core = 4
IS = LH * DH      # inner slice per core = 256
LN_EPS = 1e-5

_CACHE = {}


def _build(reps: int = 1, loop_reps: int | None = None, use_bias: bool = True):
    nc = bacc.Bacc("TRN2", target_bir_lowering=False, debug=False,
                   num_devices=N_CORES)

    xq = nc.dram_tensor("xq", [NQ, DIM], F32, kind="ExternalInput").ap()
    xk = nc.dram_tensor("xk", [NK, DIM], F32, kind="ExternalInput").ap()
    xv = nc.dram_tensor("xv", [NK, DIM], F32, kind="ExternalInput").ap()
    wqe = nc.dram_tensor("wqe", [128, 4, IS], BF16, kind="ExternalInput").ap()
    wout = nc.dram_tensor("wout", [128, 2, DIM], BF16, kind="ExternalInput").ap()
    wbv = nc.dram_tensor("wbv", [1, IS], BF16, kind="ExternalInput").ap()
    sel1 = nc.dram_tensor("sel1", [128, 2], BF16, kind="ExternalInput").ap()
    selk = nc.dram_tensor("selk", [128, 2, LH], BF16, kind="ExternalInput").ap()
    ones = nc.dram_tensor("ones", [1, DIM], BF16, kind="ExternalInput").ap()
    out_d = nc.dram_tensor("out", [NQ, DIM], F32, kind="ExternalOutput").ap()

    NQT, NKT = NQ // 128, NK // 128    # 8, 16 row tiles
    QC = NQ // 512                     # 2 q chunks
    KC = NK // 512                     # 4 k chunks

    with tile.TileContext(nc) as tc:
        with (
            tc.tile_pool(name="pc", bufs=1) as pc,
            tc.tile_pool(name="pin", bufs=2) as pin,
            tc.tile_pool(name="pst", bufs=12) as pst,
            tc.tile_pool(name="psq", bufs=2) as psq,
            tc.tile_pool(name="pz", bufs=8) as pz,
            tc.tile_pool(name="pper", bufs=1) as pper,
            tc.tile_pool(name="patt", bufs=18) as patt,
            tc.tile_pool(name="pfin", bufs=1) as pfin,
            tc.tile_pool(name="pbig", bufs=3, space="PSUM") as pbig,
            tc.tile_pool(name="pacc", bufs=1, space="PSUM") as pacc,
            tc.tile_pool(name="psm", bufs=1, space="PSUM") as psm,
        ):
            # ---- constants ----
            wqe_sb = pc.tile([128, 4, IS], BF16)
            wout_sb = pc.tile([128, 2, DIM], BF16)
            wb_sb = pc.tile([1, IS], BF16)
            sel1_sb = pc.tile([128, 2], BF16)
            selk_sb = pc.tile([128, 2, LH], BF16)
            ones_sb = pc.tile([1, DIM], BF16)
            eps_sb = pc.tile([128, 1], F32)
            nc.sync.dma_start(out=wqe_sb, in_=wqe)
            nc.sync.dma_start(out=wout_sb, in_=wout)
            nc.sync.dma_start(out=wb_sb, in_=wbv)
            nc.sync.dma_start(out=sel1_sb, in_=sel1)
            nc.sync.dma_start(out=selk_sb, in_=selk)
            nc.sync.dma_start(out=ones_sb, in_=ones)
            nc.vector.memset(eps_sb, LN_EPS)

            def emit_body():
                # ---- persistent activations ----
                zqT = pper.tile([128, 4, NQ], BF16, tag="zqT")
                zkT = pper.tile([128, 4, NK], BF16, tag="zkT")
                zvT = pper.tile([128, 4, NK], BF16, tag="zvT")
                kp = pper.tile([128, 2, NK], BF16, tag="kp")
                qp = pper.tile([128, 2, NQ], BF16, tag="qp")
                qhat = pper.tile([128, 2, NQ], BF16, tag="qhat")
                vp = pper.tile([128, NKT, IS], BF16, tag="vp")
                vpT = pper.tile([128, 2, NK], BF16, tag="vpT")
                outT = pper.tile([128, 2, NQ], BF16, tag="outT")
                kn_inv = pper.tile([128, NKT, LH], F32, tag="kn_inv")
                qn_inv = pper.tile([1, LH, NQ], BF16, tag="qn_inv")

                def ln_group(x_dram, g, zt):
                    # 512 tokens: load, stats, normalize to bf16, transpose
                    x_sb = pin.tile([128, 4, DIM], F32, tag="x")
                    src = x_dram[g * 512:(g + 1) * 512, :].rearrange(
                        "(a p) d -> p a d", p=128)
                    nc.gpsimd.dma_start(out=x_sb, in_=src)
                    mv4 = pst.tile([128, 4, 2], F32, tag="mv4")
                    for a in range(4):
                        st = pst.tile([128, 6], F32, tag="st")
                        nc.vector.bn_stats(out=st[:], in_=x_sb[:, a, :])
                        nc.vector.bn_aggr(out=mv4[:, a, :], in_=st[:])
                    sd4 = pst.tile([128, 4], F32, tag="sd4")
                    nc.scalar.activation(out=sd4[:], in_=mv4[:, :, 1], func=AF.Sqrt,
                                         bias=eps_sb[:], scale=1.0)
                    rstd4 = pst.tile([128, 4], F32, tag="rstd4")
                    nc.vector.reciprocal(out=rstd4[:], in_=sd4[:])
                    mr4 = pst.tile([128, 4], F32, tag="mr4")
                    nc.vector.tensor_mul(out=mr4[:], in0=mv4[:, :, 0], in1=rstd4[:])
                    for a in range(4):
                        nt = g * 4 + a
                        z = pz.tile([128, DIM], BF16, tag="z")
                        nc.gpsimd.tensor_scalar(out=z[:], in0=x_sb[:, a, :],
                                                scalar1=rstd4[:, a:a + 1],
                                                scalar2=mr4[:, a:a + 1],
                                                op0=MUL, op1=mybir.AluOpType.subtract)
                        nc.sync.dma_start_transpose(
                            out=zt[:, :, nt * 128:(nt + 1) * 128], in_=z[:])

                def proj_chunk(zt, ch, pt, want_sq=True):
                    # both t-halves of one 512-col chunk into a 2-bank psum;
                    # returns a transient squares tile for the norm matmuls
                    ps = pbig.tile([128, 2, 512], F32, tag="mm2")
                    for t in range(2):
                        if use_bias:
                            nc.tensor.matmul(ps[:, t, :],
                                             wb_sb[0:1, t * 128:(t + 1) * 128],
                                             ones_sb[0:1, :], start=True, stop=False,
                                             skip_group_check=(t == 1))
                        for d in range(4):
                            nc.tensor.matmul(ps[:, t, :],
                                             wqe_sb[:, d, t * 128:(t + 1) * 128],
                                             zt[:, d, ch * 512:(ch + 1) * 512],
                                             start=(not use_bias and d == 0),
                                             stop=(d == 3),
                                             skip_group_check=(t == 1))
                    sl = slice(ch * 512, (ch + 1) * 512)
                    nc.scalar.copy(out=pt[:, :, sl], in_=ps[:])
                    if not want_sq:
                        return None
                    sq2 = psq.tile([128, 2, 512], BF16, tag="sq2")
                    nc.vector.tensor_mul(out=sq2[:], in0=pt[:, :, sl],
                                         in1=pt[:, :, sl])
                    return sq2

                def vproj():
                    # project v in transposed (row-major) form like k, then
                    # flip each 512-col chunk into vp's [kv, inner] layout
                    for ch in range(KC):
                        proj_chunk(zvT, ch, vpT, want_sq=False)
                        for t2 in range(2):
                            nc.sync.dma_start_transpose(
                                out=vp[:, 4 * ch:4 * ch + 4,
                                       t2 * 128:(t2 + 1) * 128],
                                in_=vpT[:, t2, ch * 512:(ch + 1) * 512])

                kps = None

                def knorm_chunk(ch, sq2):
                    # this chunk's 4 kv-tiles into the shared norm psum bank
                    nonlocal kps
                    if kps is None:
                        kps = psm.tile([128, NKT, LH], F32, tag="sm")
                    for kl in range(4):
                        kt = ch * 4 + kl
                        for t in range(2):
                            nc.tensor.matmul(kps[:, kt, :],
                                             sq2[:, t, kl * 128:(kl + 1) * 128],
                                             selk_sb[:, t, :], start=(t == 0),
                                             stop=(t == 1), skip_group_check=(kt > 0))

                def knorm_finish():
                    sqk = pst.tile([128, NKT, LH], F32, tag="sqk")
                    nc.scalar.activation(out=sqk[:], in_=kps[:], func=AF.Sqrt)
                    nc.vector.reciprocal(out=kn_inv[:], in_=sqk[:])

                def qnorm_qhat(t, c, sq2):
                    for j in range(2):
                        h = 2 * t + j
                        qnst = pbig.tile([128, 2, 512], F32, tag="mm2")
                        qns = qnst[0:1, 0, :]
                        nc.tensor.matmul(qns, sel1_sb[:, j:j + 1], sq2[:, t, :],
                                         start=True, stop=True)
                        sq = psq.tile([1, 512], F32, tag="sqq")
                        nc.scalar.activation(out=sq[:], in_=qns, func=AF.Sqrt)
                        with nc.allow_low_precision(reason="qn_inv bf16 scale, ~0.4% ok"):
                            nc.vector.reciprocal(
                                out=qn_inv[0:1, h, c * 512:(c + 1) * 512], in_=sq[:])
                    bc = pbig.tile([128, 2, 512], F32, tag="mm2")
                    nc.tensor.matmul(bc[0:64, 0, :], ones_sb[0:1, 0:64],
                                     qn_inv[0:1, 2 * t, c * 512:(c + 1) * 512],
                                     start=True, stop=True)
                    nc.tensor.matmul(bc[64:128, 0, :], ones_sb[0:1, 0:64],
                                     qn_inv[0:1, 2 * t + 1, c * 512:(c + 1) * 512],
                                     start=True, stop=True, skip_group_check=True)
                    nc.vector.tensor_mul(out=qhat[:, t, c * 512:(c + 1) * 512],
                                         in0=bc[:, 0, :],
                                         in1=qp[:, t, c * 512:(c + 1) * 512])

                # ---- attention blocks: scores into 2-bank psum pairs, one
                # broadcast-view DVE op scales+narrows 1024 cols at a time.
                # accum of block b-1 is interleaved into scores of block b at
                # pair granularity so 16 attn buffers suffice (the slot a
                # scale is about to overwrite was read one PE step earlier).
                STEPS = [(j2, jj) for j2 in range(NKT // 2) for jj in range(2)]

                def emit_score_pair(p, c, j2, jj, si):
                    h = 2 * p + jj
                    sp = pbig.tile([128, 2, 512], F32, tag="mm2")
                    for u in range(2):
                        j = 2 * j2 + u
                        nc.tensor.matmul(
                            sp[:, u, :],
                            kp[jj * 64:(jj + 1) * 64, p, j * 128:(j + 1) * 128],
                            qhat[jj * 64:(jj + 1) * 64, p, c * 512:(c + 1) * 512],
                            start=True, stop=True, skip_group_check=(u == 1))
                    at = patt.tile([128, 2, 512], BF16, tag="attn")
                    if si % 8 < 5:
                        # one merged DVE op: per-k-row scale via broadcast view
                        nc.vector.tensor_mul(
                            out=at[:], in0=sp[:],
                            in1=kn_inv[:, 2 * j2:2 * j2 + 2, h:h + 1].to_broadcast(
                                [128, 2, 512]))
                    else:
                        for u in range(2):
                            nc.scalar.mul(out=at[:, u, :], in_=sp[:, u, :],
                                          mul=kn_inv[:, 2 * j2 + u, h:h + 1])
                    return at

                def emit_accum_pair(p, ops, at, j2, jj, si):
                    h = 2 * p + jj
                    for u in range(2):
                        nc.tensor.matmul(
                            ops[jj * 64:(jj + 1) * 64, :],
                            vp[:, 2 * j2 + u, h * 64:(h + 1) * 64],
                            at[:, u, :],
                            start=(si == 0 and u == 0) if jj == 0 else
                                  (si == 1 and u == 0),
                            stop=(si == len(STEPS) - 2 and u == 1) if jj == 0 else
                                 (si == len(STEPS) - 1 and u == 1),
                            skip_group_check=(jj == 1))

                # ---- emission schedule ----
                for g in range(4):
                    ln_group(xk, g, zkT)
                for g in range(2):
                    ln_group(xq, g, zqT)
                for ch in range(KC):
                    sq2 = proj_chunk(zkT, ch, kp)
                    knorm_chunk(ch, sq2)
                knorm_finish()
                for c in range(QC):
                    sq2 = proj_chunk(zqT, c, qp)
                    for t in range(2):
                        qnorm_qhat(t, c, sq2)
                for g in range(4):
                    ln_group(xv, g, zvT)

                blocks = [(p, c) for p in range(2) for c in range(QC)]

                # block 0 scores stand alone; v-projection slots in while its
                # scales drain, then blocks pipeline at pair granularity.
                p0, c0 = blocks[0]
                prev_tiles = [emit_score_pair(p0, c0, j2, jj, si)
                              for si, (j2, jj) in enumerate(STEPS)]
                prev_p, prev_c = p0, c0
                vproj()
                for (p, c) in blocks[1:]:
                    ops = pacc.tile([128, 512], F32, tag="acc")
                    cur = []
                    for si, (j2, jj) in enumerate(STEPS):
                        cur.append(emit_score_pair(p, c, j2, jj, si))
                        emit_accum_pair(prev_p, ops, prev_tiles[si], j2, jj, si)
                    nc.scalar.copy(out=outT[:, prev_p, prev_c * 512:(prev_c + 1) * 512],
                                   in_=ops[:])
                    prev_tiles, prev_p, prev_c = cur, p, c
                def final_pair(m2):
                    # two row-tiles of the output projection share a psum pair
                    fp = pbig.tile([128, 2, 512], F32, tag="mm2")
                    for u in range(2):
                        m = 2 * m2 + u
                        for t in range(2):
                            nc.tensor.matmul(fp[:, u, :],
                                             outT[:, t, m * 128:(m + 1) * 128],
                                             wout_sb[:, t, :], start=(t == 0),
                                             stop=(t == 1), skip_group_check=(u == 1))
                    o_sb = pfin.tile([128, 2, DIM], F32, tag="o")
                    nc.scalar.copy(out=o_sb[:], in_=fp[:])
                    dst = out_d[m2 * 256:(m2 + 1) * 256, :].rearrange(
                        "(a p) d -> p a d", p=128)
                    nc.gpsimd.dma_start(out=dst, in_=o_sb[:])

                # q-chunk 0 of outT is complete once block (1,0)'s accum landed
                # (copied inside the last loop iteration) -> its final
                # projection overlaps the last block's scores/accum.
                final_pair(0)
                final_pair(1)
                ops = pacc.tile([128, 512], F32, tag="acc")
                for si, (j2, jj) in enumerate(STEPS):
                    emit_accum_pair(prev_p, ops, prev_tiles[si], j2, jj, si)
                nc.scalar.copy(out=outT[:, prev_p, prev_c * 512:(prev_c + 1) * 512],
                               in_=ops[:])
                final_pair(2)
                final_pair(3)

            n_iter = reps if loop_reps is None else loop_reps
            if n_iter == 0:      # straight-line build (offline sim only)
                emit_body()
            else:
                with tc.For_i(0, n_iter, 1) as _i:
                    emit_body()

    nc.compile()
    return nc


def _get_nc(reps: int = 1, loop_reps=None, use_bias: bool = True):
    key = (reps, loop_reps, use_bias)
    if key not in _CACHE:
        _CACHE[key] = _build(reps, loop_reps, use_bias)
    return _CACHE[key]


def _host_prep(q, k, v, ln_gamma, ln_beta, W_qkv, W_out):
    q = np.asarray(q, np.float32)
    k = np.asarray(k, np.float32)
    v = np.asarray(v, np.float32)
    g = np.asarray(ln_gamma, np.float32)
    bt = np.asarray(ln_beta, np.float32)
    Wq = np.asarray(W_qkv, np.float32)[:, :HEADS * DH]
    Wo = np.asarray(W_out, np.float32)

    sel1 = np.zeros((128, 2), np.float32)
    sel1[0:64, 0] = 1.0
    sel1[64:128, 1] = 1.0
    selk = np.zeros((128, 2, LH), np.float32)
    for t in range(2):
        for j in range(2):
            selk[j * 64:(j + 1) * 64, t, 2 * t + j] = 1.0
    ones = np.ones((1, DIM), np.float32)

    bf = ml_dtypes.bfloat16
    in_maps = []
    for core in range(N_CORES):
        b, grp = core // HG, core % HG
        csl = slice(grp * IS, (grp + 1) * IS)
        Wq_g = Wq[:, csl]
        wqe = np.ascontiguousarray(
            (g[:, None] * Wq_g).reshape(4, 128, IS).transpose(1, 0, 2)).astype(bf)
        wb = (bt @ Wq_g).reshape(1, IS).astype(bf)
        wo = np.ascontiguousarray(
            Wo[csl, :].reshape(2, 128, DIM).transpose(1, 0, 2)).astype(bf)
        in_maps.append({
            "xq": np.ascontiguousarray(q[b]),
            "xk": np.ascontiguousarray(k[b]),
            "xv": np.ascontiguousarray(v[b]),
            "wqe": wqe,
            "wout": wo,
            "wbv": wb,
            "sel1": sel1.astype(bf),
            "selk": selk.astype(bf),
            "ones": ones.astype(bf),
        })
    return in_maps


def kernel(q, k, v, ln_gamma, ln_beta, W_qkv, W_out, b_out):
    in_maps = _host_prep(q, k, v, ln_gamma, ln_beta, W_qkv, W_out)
    use_bias = any(np.any(m["wbv"]) for m in in_maps)
    nc = _get_nc(1, use_bias=use_bias)
    res = bass_utils.run_bass_kernel_spmd(nc, in_maps, core_ids=list(range(N_CORES)))
    b_out = np.asarray(b_out, np.float32)
    out = np.empty((B, NQ, DIM), np.float32)
    for b in range(B):
        out[b] = res.results[b * HG]["out"] + res.results[b * HG + 1]["out"] + b_out
    return out


# revision 48
# speedup vs baseline: 1.1343x; 1.1343x over previous
"""Cosine attention (nn_CosineAttention) Trainium2 Bass kernel.

Sharding: 8 cores = 4 batches x 2 head-groups (4 heads each).
Each core: LN(q/k/v) -> project with gamma/beta folded into weights ->
cosine attention for its 4 heads -> partial output projection with its
W_out row-slice. Host sums the two head-group partials per batch and
adds b_out.

Math notes:
 - LN(x) @ Wq == ((x-mu)*rstd) @ (gamma[:,None]*Wq) + beta@Wq : exact fold.
 - attn = (qh.kh)/(|qh||kh|+eps): eps folded (rel err ~1e-10); q-side norm
   applied to q-hat before scores, k-side norm applied on the score
   psum->sbuf copy via a broadcast-view multiply (score_T rows are k rows).

Schedule: emission order software-pipelines the phases so the PE stays fed:
k-LN -> q-LN -> k-proj(+norm chunks) -> q-proj/q-norms/qhat -> v-LN ->
scores(0) -> v-proj -> pair-interleaved scores(b)/accum(b-1) pipeline ->
final projection (first half overlapped with the last block).
Score PSUM tiles pair two k-tiles (2 banks); 5/8 of the scale+narrow ops
run as one merged DVE op per pair (per-k-row scale via a stride-0
broadcast view), the rest as per-bank scalar-engine multiplies. Timing
reps run the body in a hardware For_i loop so the NEFF stays constant
across rep counts and the per-rep wall-clock delta isolates on-device
execution.
"""

import sys

sys.path.insert(0, "/opt/trn_rl_repo")

import numpy as np
import ml_dtypes

import concourse.bass as bass  # noqa: F401  (AP helpers)
import concourse.mybir as mybir
import concourse.tile as tile
from concourse import bacc, bass_utils

BF16 = mybir.dt.bfloat16
F32 = mybir.dt.float32
AF = mybir.ActivationFunctionType
MUL = mybir.AluOpType.mult
ADD = mybir.AluOpType.add

HEADS = 8
DH = 64
DIM = 512
NQ = 1024
NK = 2048
B = 4
N_CORES = 8
HG = 2            # head groups (cores per batch)
LH = HEADS // HG  # local heads per core = 4
IS = LH * DH      # inner slice per core = 256
LN_EPS = 1e-5

_CACHE = {}


def _build(reps: int = 1, loop_reps: int | None = None, use_bias: bool = True):
    nc = bacc.Bacc("TRN2", target_bir_lowering=False, debug=False,
                   num_devices=N_CORES)

    xq = nc.dram_tensor("xq", [NQ, DIM], F32, kind="ExternalInput").ap()
    xk = nc.dram_tensor("xk", [NK, DIM], F32, kind="ExternalInput").ap()
    xv = nc.dram_tensor("xv", [NK, DIM], F32, kind="ExternalInput").ap()
    wqe = nc.dram_tensor("wqe", [128, 4, IS], BF16, kind="ExternalInput").ap()
    wout = nc.dram_tensor("wout", [128, 2, DIM], BF16, kind="ExternalInput").ap()
    wbv = nc.dram_tensor("wbv", [1, IS], BF16, kind="ExternalInput").ap()
    sel1 = nc.dram_tensor("sel1", [128, 2], BF16, kind="ExternalInput").ap()
    selk = nc.dram_tensor("selk", [128, 2, LH], BF16, kind="ExternalInput").ap()
    ones = nc.dram_tensor("ones", [1, DIM], BF16, kind="ExternalInput").ap()
    out_d = nc.dram_tensor("out", [NQ, DIM], F32, kind="ExternalOutput").ap()

    NQT, NKT = NQ // 128, NK // 128    # 8, 16 row tiles
    QC = NQ // 512                     # 2 q chunks
    KC = NK // 512                     # 4 k chunks

    with tile.TileContext(nc) as tc:
        with (
            tc.tile_pool(name="pc", bufs=1) as pc,
            tc.tile_pool(name="pin", bufs=2) as pin,
            tc.tile_pool(name="pst", bufs=12) as pst,
            tc.tile_pool(name="psq", bufs=2) as psq,
            tc.tile_pool(name="pz", bufs=8) as pz,
            tc.tile_pool(name="pper", bufs=1) as pper,
            tc.tile_pool(name="patt", bufs=18) as patt,
            tc.tile_pool(name="pfin", bufs=1) as pfin,
            tc.tile_pool(name="pbig", bufs=2, space="PSUM") as pbig,
            tc.tile_pool(name="pacc", bufs=2, space="PSUM") as pacc,
            tc.tile_pool(name="psm", bufs=1, space="PSUM") as psm,
        ):
            # ---- constants ----
            wqe_sb = pc.tile([128, 4, IS], BF16)
            wout_sb = pc.tile([128, 2, DIM], BF16)
            wb_sb = pc.tile([1, IS], BF16)
            sel1_sb = pc.tile([128, 2], BF16)
            selk_sb = pc.tile([128, 2, LH], BF16)
            ones_sb = pc.tile([1, DIM], BF16)
            eps_sb = pc.tile([128, 1], F32)
            nc.sync.dma_start(out=wqe_sb, in_=wqe)
            nc.sync.dma_start(out=wout_sb, in_=wout)
            nc.sync.dma_start(out=wb_sb, in_=wbv)
            nc.sync.dma_start(out=sel1_sb, in_=sel1)
            nc.sync.dma_start(out=selk_sb, in_=selk)
            nc.sync.dma_start(out=ones_sb, in_=ones)
            nc.vector.memset(eps_sb, LN_EPS)

            def emit_body():
                # ---- persistent activations ----
                zqT = pper.tile([128, 4, NQ], BF16, tag="zqT")
                zkT = pper.tile([128, 4, NK], BF16, tag="zkT")
                zvT = pper.tile([128, 4, NK], BF16, tag="zvT")
                kp = pper.tile([128, 2, NK], BF16, tag="kp")
                qp = pper.tile([128, 2, NQ], BF16, tag="qp")
                qhat = pper.tile([128, 2, NQ], BF16, tag="qhat")
                vp = pper.tile([128, NKT, IS], BF16, tag="vp")
                outT = pper.tile([128, 2, NQ], BF16, tag="outT")
                kn_inv = pper.tile([128, NKT, LH], F32, tag="kn_inv")
                qn_inv = pper.tile([1, LH, NQ], BF16, tag="qn_inv")

                def ln_group(x_dram, g, zt):
                    # 512 tokens: load, stats, normalize to bf16, transpose
                    x_sb = pin.tile([128, 4, DIM], F32, tag="x")
                    src = x_dram[g * 512:(g + 1) * 512, :].rearrange(
                        "(a p) d -> p a d", p=128)
                    nc.gpsimd.dma_start(out=x_sb, in_=src)
                    mv4 = pst.tile([128, 4, 2], F32, tag="mv4")
                    for a in range(4):
                        st = pst.tile([128, 6], F32, tag="st")
                        nc.vector.bn_stats(out=st[:], in_=x_sb[:, a, :])
                        nc.vector.bn_aggr(out=mv4[:, a, :], in_=st[:])
                    sd4 = pst.tile([128, 4], F32, tag="sd4")
                    nc.scalar.activation(out=sd4[:], in_=mv4[:, :, 1], func=AF.Sqrt,
                                         bias=eps_sb[:], scale=1.0)
                    rstd4 = pst.tile([128, 4], F32, tag="rstd4")
                    nc.vector.reciprocal(out=rstd4[:], in_=sd4[:])
                    mr4 = pst.tile([128, 4], F32, tag="mr4")
                    nc.vector.tensor_mul(out=mr4[:], in0=mv4[:, :, 0], in1=rstd4[:])
                    for a in range(4):
                        nt = g * 4 + a
                        z = pz.tile([128, DIM], BF16, tag="z")
                        nc.gpsimd.tensor_scalar(out=z[:], in0=x_sb[:, a, :],
                                                scalar1=rstd4[:, a:a + 1],
                                                scalar2=mr4[:, a:a + 1],
                                                op0=MUL, op1=mybir.AluOpType.subtract)
                        nc.sync.dma_start_transpose(
                            out=zt[:, :, nt * 128:(nt + 1) * 128], in_=z[:])

                def proj_chunk(zt, ch, pt, want_sq=True):
                    # both t-halves of one 512-col chunk into a 2-bank psum;
                    # returns a transient squares tile for the norm matmuls
                    ps = pbig.tile([128, 2, 512], F32, tag="mm2")
                    for t in range(2):
                        if use_bias:
                            nc.tensor.matmul(ps[:, t, :],
                                             wb_sb[0:1, t * 128:(t + 1) * 128],
                                             ones_sb[0:1, :], start=True, stop=False,
                                             skip_group_check=(t == 1))
                        for d in range(4):
                            nc.tensor.matmul(ps[:, t, :],
                                             wqe_sb[:, d, t * 128:(t + 1) * 128],
                                             zt[:, d, ch * 512:(ch + 1) * 512],
                                             start=(not use_bias and d == 0),
                                             stop=(d == 3),
                                             skip_group_check=(t == 1))
                    sl = slice(ch * 512, (ch + 1) * 512)
                    nc.scalar.copy(out=pt[:, :, sl], in_=ps[:])
                    if not want_sq:
                        return None
                    sq2 = psq.tile([128, 2, 512], BF16, tag="sq2")
                    nc.vector.tensor_mul(out=sq2[:], in0=pt[:, :, sl],
                                         in1=pt[:, :, sl])
                    return sq2

                def vproj():
                    # 2 v row-tiles (256 cols each) per psum pair, bank each
                    for g2 in range(NKT // 2):
                        ps = pbig.tile([128, 2, 512], F32, tag="mm2")
                        for u in range(2):
                            nt = g2 * 2 + u
                            if use_bias:
                                nc.tensor.matmul(ps[:, u, 0:IS], ones_sb[0:1, 0:128],
                                                 wb_sb[0:1, :], start=True, stop=False,
                                                 skip_group_check=(u == 1))
                            for d in range(4):
                                nc.tensor.matmul(ps[:, u, 0:IS],
                                                 zvT[:, d, nt * 128:(nt + 1) * 128],
                                                 wqe_sb[:, d, :],
                                                 start=(not use_bias and d == 0),
                                                 stop=(d == 3),
                                                 skip_group_check=(u == 1))
                        nc.vector.tensor_copy(out=vp[:, g2 * 2:g2 * 2 + 2, :],
                                              in_=ps[:, :, 0:IS])

                kps = None

                def knorm_chunk(ch, sq2):
                    # this chunk's 4 kv-tiles into the shared norm psum bank
                    nonlocal kps
                    if kps is None:
                        kps = psm.tile([128, NKT, LH], F32, tag="sm")
                    for kl in range(4):
                        kt = ch * 4 + kl
                        for t in range(2):
                            nc.tensor.matmul(kps[:, kt, :],
                                             sq2[:, t, kl * 128:(kl + 1) * 128],
                                             selk_sb[:, t, :], start=(t == 0),
                                             stop=(t == 1), skip_group_check=(kt > 0))

                def knorm_finish():
                    sqk = pst.tile([128, NKT, LH], F32, tag="sqk")
                    nc.scalar.activation(out=sqk[:], in_=kps[:], func=AF.Sqrt)
                    nc.vector.reciprocal(out=kn_inv[:], in_=sqk[:])

                def qnorm_qhat(t, c, sq2):
                    for j in range(2):
                        h = 2 * t + j
                        qns = psm.tile([1, 512], F32, tag="smq")
                        nc.tensor.matmul(qns[:], sel1_sb[:, j:j + 1], sq2[:, t, :],
                                         start=True, stop=True)
                        sq = psq.tile([1, 512], F32, tag="sqq")
                        nc.scalar.activation(out=sq[:], in_=qns[:], func=AF.Sqrt)
                        with nc.allow_low_precision(reason="qn_inv bf16 scale, ~0.4% ok"):
                            nc.vector.reciprocal(
                                out=qn_inv[0:1, h, c * 512:(c + 1) * 512], in_=sq[:])
                    bc = pbig.tile([128, 2, 512], F32, tag="mm2")
                    nc.tensor.matmul(bc[0:64, 0, :], ones_sb[0:1, 0:64],
                                     qn_inv[0:1, 2 * t, c * 512:(c + 1) * 512],
                                     start=True, stop=True)
                    nc.tensor.matmul(bc[64:128, 0, :], ones_sb[0:1, 0:64],
                                     qn_inv[0:1, 2 * t + 1, c * 512:(c + 1) * 512],
                                     start=True, stop=True, skip_group_check=True)
                    nc.vector.tensor_mul(out=qhat[:, t, c * 512:(c + 1) * 512],
                                         in0=bc[:, 0, :],
                                         in1=qp[:, t, c * 512:(c + 1) * 512])

                # ---- attention blocks: scores into 2-bank psum pairs, one
                # broadcast-view DVE op scales+narrows 1024 cols at a time.
                # accum of block b-1 is interleaved into scores of block b at
                # pair granularity so 16 attn buffers suffice (the slot a
                # scale is about to overwrite was read one PE step earlier).
                STEPS = [(j2, jj) for j2 in range(NKT // 2) for jj in range(2)]

                def emit_score_pair(p, c, j2, jj, si):
                    h = 2 * p + jj
                    sp = pbig.tile([128, 2, 512], F32, tag="mm2")
                    for u in range(2):
                        j = 2 * j2 + u
                        nc.tensor.matmul(
                            sp[:, u, :],
                            kp[jj * 64:(jj + 1) * 64, p, j * 128:(j + 1) * 128],
                            qhat[jj * 64:(jj + 1) * 64, p, c * 512:(c + 1) * 512],
                            start=True, stop=True, skip_group_check=(u == 1))
                    at = patt.tile([128, 2, 512], BF16, tag="attn")
                    if si % 8 < 5:
                        # one merged DVE op: per-k-row scale via broadcast view
                        nc.vector.tensor_mul(
                            out=at[:], in0=sp[:],
                            in1=kn_inv[:, 2 * j2:2 * j2 + 2, h:h + 1].to_broadcast(
                                [128, 2, 512]))
                    else:
                        for u in range(2):
                            nc.scalar.mul(out=at[:, u, :], in_=sp[:, u, :],
                                          mul=kn_inv[:, 2 * j2 + u, h:h + 1])
                    return at

                def emit_accum_pair(p, ops, at, j2, jj, si):
                    h = 2 * p + jj
                    for u in range(2):
                        nc.tensor.matmul(
                            ops[jj * 64:(jj + 1) * 64, :],
                            vp[:, 2 * j2 + u, h * 64:(h + 1) * 64],
                            at[:, u, :],
                            start=(si == 0 and u == 0) if jj == 0 else
                                  (si == 1 and u == 0),
                            stop=(si == len(STEPS) - 2 and u == 1) if jj == 0 else
                                 (si == len(STEPS) - 1 and u == 1),
                            skip_group_check=(jj == 1))

                # ---- emission schedule ----
                for g in range(4):
                    ln_group(xk, g, zkT)
                for g in range(2):
                    ln_group(xq, g, zqT)
                for ch in range(KC):
                    sq2 = proj_chunk(zkT, ch, kp)
                    knorm_chunk(ch, sq2)
                knorm_finish()
                for c in range(QC):
                    sq2 = proj_chunk(zqT, c, qp)
                    for t in range(2):
                        qnorm_qhat(t, c, sq2)
                for g in range(4):
                    ln_group(xv, g, zvT)

                blocks = [(p, c) for p in range(2) for c in range(QC)]

                # block 0 scores stand alone; v-projection slots in while its
                # scales drain, then blocks pipeline at pair granularity.
                p0, c0 = blocks[0]
                prev_tiles = [emit_score_pair(p0, c0, j2, jj, si)
                              for si, (j2, jj) in enumerate(STEPS)]
                prev_p, prev_c = p0, c0
                vproj()
                for (p, c) in blocks[1:]:
                    ops = pacc.tile([128, 512], F32, tag="acc")
                    cur = []
                    for si, (j2, jj) in enumerate(STEPS):
                        cur.append(emit_score_pair(p, c, j2, jj, si))
                        emit_accum_pair(prev_p, ops, prev_tiles[si], j2, jj, si)
                    nc.scalar.copy(out=outT[:, prev_p, prev_c * 512:(prev_c + 1) * 512],
                                   in_=ops[:])
                    prev_tiles, prev_p, prev_c = cur, p, c
                def final_pair(m2):
                    # two row-tiles of the output projection share a psum pair
                    fp = pbig.tile([128, 2, 512], F32, tag="mm2")
                    for u in range(2):
                        m = 2 * m2 + u
                        for t in range(2):
                            nc.tensor.matmul(fp[:, u, :],
                                             outT[:, t, m * 128:(m + 1) * 128],
                                             wout_sb[:, t, :], start=(t == 0),
                                             stop=(t == 1), skip_group_check=(u == 1))
                    o_sb = pfin.tile([128, 2, DIM], F32, tag="o")
                    nc.scalar.copy(out=o_sb[:], in_=fp[:])
                    dst = out_d[m2 * 256:(m2 + 1) * 256, :].rearrange(
                        "(a p) d -> p a d", p=128)
                    nc.gpsimd.dma_start(out=dst, in_=o_sb[:])

                # q-chunk 0 of outT is complete once block (1,0)'s accum landed
                # (copied inside the last loop iteration) -> its final
                # projection overlaps the last block's scores/accum.
                final_pair(0)
                final_pair(1)
                ops = pacc.tile([128, 512], F32, tag="acc")
                for si, (j2, jj) in enumerate(STEPS):
                    emit_accum_pair(prev_p, ops, prev_tiles[si], j2, jj, si)
                nc.scalar.copy(out=outT[:, prev_p, prev_c * 512:(prev_c + 1) * 512],
                               in_=ops[:])
                final_pair(2)
                final_pair(3)

            n_iter = reps if loop_reps is None else loop_reps
            if n_iter == 0:      # straight-line build (offline sim only)
                emit_body()
            else:
                with tc.For_i(0, n_iter, 1) as _i:
                    emit_body()

    nc.compile()
    return nc


def _get_nc(reps: int = 1, loop_reps=None, use_bias: bool = True):
    key = (reps, loop_reps, use_bias)
    if key not in _CACHE:
        _CACHE[key] = _build(reps, loop_reps, use_bias)
    return _CACHE[key]


def _host_prep(q, k, v, ln_gamma, ln_beta, W_qkv, W_out):
    q = np.asarray(q, np.float32)
    k = np.asarray(k, np.float32)
    v = np.asarray(v, np.float32)
    g = np.asarray(ln_gamma, np.float32)
    bt = np.asarray(ln_beta, np.float32)
    Wq = np.asarray(W_qkv, np.float32)[:, :HEADS * DH]
    Wo = np.asarray(W_out, np.float32)

    sel1 = np.zeros((128, 2), np.float32)
    sel1[0:64, 0] = 1.0
    sel1[64:128, 1] = 1.0
    selk = np.zeros((128, 2, LH), np.float32)
    for t in range(2):
        for j in range(2):
            selk[j * 64:(j + 1) * 64, t, 2 * t + j] = 1.0
    ones = np.ones((1, DIM), np.float32)

    bf = ml_dtypes.bfloat16
    in_maps = []
    for core in range(N_CORES):
        b, grp = core // HG, core % HG
        csl = slice(grp * IS, (grp + 1) * IS)
        Wq_g = Wq[:, csl]
        wqe = np.ascontiguousarray(
            (g[:, None] * Wq_g).reshape(4, 128, IS).transpose(1, 0, 2)).astype(bf)
        wb = (bt @ Wq_g).reshape(1, IS).astype(bf)
        wo = np.ascontiguousarray(
            Wo[csl, :].reshape(2, 128, DIM).transpose(1, 0, 2)).astype(bf)
        in_maps.append({
            "xq": np.ascontiguousarray(q[b]),
            "xk": np.ascontiguousarray(k[b]),
            "xv": np.ascontiguousarray(v[b]),
            "wqe": wqe,
            "wout": wo,
            "wbv": wb,
            "sel1": sel1.astype(bf),
            "selk": selk.astype(bf),
            "ones": ones.astype(bf),
        })
    return in_maps


def kernel(q, k, v, ln_gamma, ln_beta, W_qkv, W_out, b_out):
    in_maps = _host_prep(q, k, v, ln_gamma, ln_beta, W_qkv, W_out)
    use_bias = any(np.any(m["wbv"]) for m in in_maps)
    nc = _get_nc(1, use_bias=use_bias)
    res = bass_utils.run_bass_kernel_spmd(nc, in_maps, core_ids=list(range(N_CORES)))
    b_out = np.asarray(b_out, np.float32)
    out = np.empty((B, NQ, DIM), np.float32)
    for b in range(B):
        out[b] = res.results[b * HG]["out"] + res.results[b * HG + 1]["out"] + b_out
    return out


# revision 55
# speedup vs baseline: 1.2435x; 1.0963x over previous
"""Cosine attention (nn_CosineAttention) Trainium2 Bass kernel.

Sharding: 8 cores = 4 batches x 2 head-groups (4 heads each).
Each core: LN(q/k/v) -> project with gamma/beta folded into weights ->
cosine attention for its 4 heads -> partial output projection with its
W_out row-slice. Host sums the two head-group partials per batch and
adds b_out.

Math notes:
 - LN(x) @ Wq == ((x-mu)*rstd) @ (gamma[:,None]*Wq) + beta@Wq : exact fold.
 - attn = (qh.kh)/(|qh||kh|+eps): eps folded (rel err ~1e-10); q-side norm
   applied to q-hat before scores, k-side norm applied on the score
   psum->sbuf copy via a broadcast-view multiply (score_T rows are k rows).

Schedule: emission order software-pipelines the phases so the PE stays fed:
k-LN -> q-LN -> k-proj(+norm chunks) -> q-proj/q-norms/qhat -> v-LN ->
scores(0) -> v-proj -> pair-interleaved scores(b)/accum(b-1) pipeline ->
final projection (first half overlapped with the last block).
Score PSUM tiles pair two k-tiles (2 banks); 5/8 of the scale+narrow ops
run as one merged DVE op per pair (per-k-row scale via a stride-0
broadcast view), the rest as per-bank scalar-engine multiplies. Timing
reps run the body in a hardware For_i loop so the NEFF stays constant
across rep counts and the per-rep wall-clock delta isolates on-device
execution.
"""

import sys

sys.path.insert(0, "/opt/trn_rl_repo")

import numpy as np
import ml_dtypes

import concourse.bass as bass  # noqa: F401  (AP helpers)
import concourse.mybir as mybir
import concourse.tile as tile
from concourse import bacc, bass_utils

BF16 = mybir.dt.bfloat16
F32 = mybir.dt.float32
AF = mybir.ActivationFunctionType
MUL = mybir.AluOpType.mult
ADD = mybir.AluOpType.add

HEADS = 8
DH = 64
DIM = 512
NQ = 1024
NK = 2048
B = 4
N_CORES = 8
HG = 2            # head groups (cores per batch)
LH = HEADS // HG  # local heads per core = 4
IS = LH * DH      # inner slice per core = 256
LN_EPS = 1e-5

_CACHE = {}


def _build(reps: int = 1, loop_reps: int | None = None, use_bias: bool = True):
    nc = bacc.Bacc("TRN2", target_bir_lowering=False, debug=False,
                   num_devices=N_CORES)

    xq = nc.dram_tensor("xq", [NQ, DIM], F32, kind="ExternalInput").ap()
    xk = nc.dram_tensor("xk", [NK, DIM], F32, kind="ExternalInput").ap()
    xv = nc.dram_tensor("xv", [NK, DIM], F32, kind="ExternalInput").ap()
    wqe = nc.dram_tensor("wqe", [128, 4, IS], BF16, kind="ExternalInput").ap()
    wout = nc.dram_tensor("wout", [128, 2, DIM], BF16, kind="ExternalInput").ap()
    wbv = nc.dram_tensor("wbv", [1, IS], BF16, kind="ExternalInput").ap()
    sel1 = nc.dram_tensor("sel1", [128, 2], BF16, kind="ExternalInput").ap()
    selk = nc.dram_tensor("selk", [128, 2, LH], BF16, kind="ExternalInput").ap()
    ones = nc.dram_tensor("ones", [1, DIM], BF16, kind="ExternalInput").ap()
    out_d = nc.dram_tensor("out", [NQ, DIM], F32, kind="ExternalOutput").ap()

    NQT, NKT = NQ // 128, NK // 128    # 8, 16 row tiles
    QC = NQ // 512                     # 2 q chunks
    KC = NK // 512                     # 4 k chunks

    with tile.TileContext(nc) as tc:
        with (
            tc.tile_pool(name="pc", bufs=1) as pc,
            tc.tile_pool(name="pin", bufs=2) as pin,
            tc.tile_pool(name="pst", bufs=12) as pst,
            tc.tile_pool(name="psq", bufs=2) as psq,
            tc.tile_pool(name="pz", bufs=8) as pz,
            tc.tile_pool(name="pper", bufs=1) as pper,
            tc.tile_pool(name="patt", bufs=18) as patt,
            tc.tile_pool(name="pfin", bufs=1) as pfin,
            tc.tile_pool(name="pbig", bufs=2, space="PSUM") as pbig,
            tc.tile_pool(name="pacc", bufs=2, space="PSUM") as pacc,
            tc.tile_pool(name="psm", bufs=1, space="PSUM") as psm,
        ):
            # ---- constants ----
            wqe_sb = pc.tile([128, 4, IS], BF16)
            wout_sb = pc.tile([128, 2, DIM], BF16)
            wb_sb = pc.tile([1, IS], BF16)
            sel1_sb = pc.tile([128, 2], BF16)
            selk_sb = pc.tile([128, 2, LH], BF16)
            ones_sb = pc.tile([1, DIM], BF16)
            eps_sb = pc.tile([128, 1], F32)
            nc.sync.dma_start(out=wqe_sb, in_=wqe)
            nc.sync.dma_start(out=wout_sb, in_=wout)
            nc.sync.dma_start(out=wb_sb, in_=wbv)
            nc.sync.dma_start(out=sel1_sb, in_=sel1)
            nc.sync.dma_start(out=selk_sb, in_=selk)
            nc.sync.dma_start(out=ones_sb, in_=ones)
            nc.vector.memset(eps_sb, LN_EPS)

            def emit_body():
                # ---- persistent activations ----
                # token-pair-major transpose layout: [dim-row, token-pair,
                # (tile-in-pair a, dim-quarter q), token] — lets one DMA
                # transpose flip 1024 columns (two 128-token tiles) at once
                zqT = pper.tile([128, NQT // 2, 8, 128], BF16, tag="zqT")
                zkT = pper.tile([128, NKT // 2, 8, 128], BF16, tag="zkT")
                zvT = pper.tile([128, NKT // 2, 8, 128], BF16, tag="zvT")
                kp = pper.tile([128, 2, NK], BF16, tag="kp")
                qp = pper.tile([128, 2, NQ], BF16, tag="qp")
                qhat = pper.tile([128, 2, NQ], BF16, tag="qhat")
                vp = pper.tile([128, NKT, IS], BF16, tag="vp")
                outT = pper.tile([128, 2, NQ], BF16, tag="outT")
                kn_inv = pper.tile([128, NKT, LH], F32, tag="kn_inv")
                qn_inv = pper.tile([1, LH, NQ], BF16, tag="qn_inv")
                # 5D views splitting the (a q) slot dim for the matmul reads
                zqr = zqT[:].rearrange("p n (a q) t -> p n a q t", a=2)
                zkr = zkT[:].rearrange("p n (a q) t -> p n a q t", a=2)
                zvr = zvT[:].rearrange("p n (a q) t -> p n a q t", a=2)

                def ln_group(x_dram, g, zt):
                    # 512 tokens: load, stats, normalize to bf16, transpose
                    x_sb = pin.tile([128, 4, DIM], F32, tag="x")
                    src = x_dram[g * 512:(g + 1) * 512, :].rearrange(
                        "(a p) d -> p a d", p=128)
                    nc.gpsimd.dma_start(out=x_sb, in_=src)
                    mv4 = pst.tile([128, 4, 2], F32, tag="mv4")
                    for a in range(4):
                        st = pst.tile([128, 6], F32, tag="st")
                        nc.vector.bn_stats(out=st[:], in_=x_sb[:, a, :])
                        nc.vector.bn_aggr(out=mv4[:, a, :], in_=st[:])
                    sd4 = pst.tile([128, 4], F32, tag="sd4")
                    nc.scalar.activation(out=sd4[:], in_=mv4[:, :, 1], func=AF.Sqrt,
                                         bias=eps_sb[:], scale=1.0)
                    rstd4 = pst.tile([128, 4], F32, tag="rstd4")
                    nc.vector.reciprocal(out=rstd4[:], in_=sd4[:])
                    mr4 = pst.tile([128, 4], F32, tag="mr4")
                    nc.vector.tensor_mul(out=mr4[:], in0=mv4[:, :, 0], in1=rstd4[:])
                    for half in range(2):
                        z2 = pz.tile([128, 2, DIM], BF16, tag="z")
                        for u in range(2):
                            a = half * 2 + u
                            nc.gpsimd.tensor_scalar(out=z2[:, u, :], in0=x_sb[:, a, :],
                                                    scalar1=rstd4[:, a:a + 1],
                                                    scalar2=mr4[:, a:a + 1],
                                                    op0=MUL,
                                                    op1=mybir.AluOpType.subtract)
                        nc.sync.dma_start_transpose(
                            out=zt[:, g * 2 + half, :, :], in_=z2[:])

                def proj_chunk(zt, ch, pt, want_sq=True):
                    # both t-halves of one 512-col chunk into a 2-bank psum;
                    # returns a transient squares tile for the norm matmuls
                    ps = pbig.tile([128, 2, 512], F32, tag="mm2")
                    for t in range(2):
                        if use_bias:
                            nc.tensor.matmul(ps[:, t, :],
                                             wb_sb[0:1, t * 128:(t + 1) * 128],
                                             ones_sb[0:1, :], start=True, stop=False,
                                             skip_group_check=(t == 1))
                        for d in range(4):
                            nc.tensor.matmul(ps[:, t, :],
                                             wqe_sb[:, d, t * 128:(t + 1) * 128],
                                             zt[:, 2 * ch:2 * ch + 2, :, d, :],
                                             start=(not use_bias and d == 0),
                                             stop=(d == 3),
                                             skip_group_check=(t == 1))
                    sl = slice(ch * 512, (ch + 1) * 512)
                    nc.scalar.copy(out=pt[:, :, sl], in_=ps[:])
                    if not want_sq:
                        return None
                    sq2 = psq.tile([128, 2, 512], BF16, tag="sq2")
                    nc.vector.tensor_mul(out=sq2[:], in0=pt[:, :, sl],
                                         in1=pt[:, :, sl])
                    return sq2

                def vproj():
                    # 2 v row-tiles (256 cols each) per psum pair, bank each
                    for g2 in range(NKT // 2):
                        ps = pbig.tile([128, 2, 512], F32, tag="mm2")
                        for u in range(2):
                            nt = g2 * 2 + u
                            if use_bias:
                                nc.tensor.matmul(ps[:, u, 0:IS], ones_sb[0:1, 0:128],
                                                 wb_sb[0:1, :], start=True, stop=False,
                                                 skip_group_check=(u == 1))
                            for d in range(4):
                                nc.tensor.matmul(ps[:, u, 0:IS],
                                                 zvr[:, nt // 2, nt % 2, d, :],
                                                 wqe_sb[:, d, :],
                                                 start=(not use_bias and d == 0),
                                                 stop=(d == 3),
                                                 skip_group_check=(u == 1))
                        nc.vector.tensor_copy(out=vp[:, g2 * 2:g2 * 2 + 2, :],
                                              in_=ps[:, :, 0:IS])

                kps = None

                def knorm_chunk(ch, sq2):
                    # this chunk's 4 kv-tiles into the shared norm psum bank
                    nonlocal kps
                    if kps is None:
                        kps = psm.tile([128, NKT, LH], F32, tag="sm")
                    for kl in range(4):
                        kt = ch * 4 + kl
                        for t in range(2):
                            nc.tensor.matmul(kps[:, kt, :],
                                             sq2[:, t, kl * 128:(kl + 1) * 128],
                                             selk_sb[:, t, :], start=(t == 0),
                                             stop=(t == 1), skip_group_check=(kt > 0))

                def knorm_finish():
                    sqk = pst.tile([128, NKT, LH], F32, tag="sqk")
                    nc.scalar.activation(out=sqk[:], in_=kps[:], func=AF.Sqrt)
                    nc.vector.reciprocal(out=kn_inv[:], in_=sqk[:])

                def qnorm_qhat(t, c, sq2):
                    for j in range(2):
                        h = 2 * t + j
                        qns = psm.tile([1, 512], F32, tag="smq")
                        nc.tensor.matmul(qns[:], sel1_sb[:, j:j + 1], sq2[:, t, :],
                                         start=True, stop=True)
                        sq = psq.tile([1, 512], F32, tag="sqq")
                        nc.scalar.activation(out=sq[:], in_=qns[:], func=AF.Sqrt)
                        with nc.allow_low_precision(reason="qn_inv bf16 scale, ~0.4% ok"):
                            nc.vector.reciprocal(
                                out=qn_inv[0:1, h, c * 512:(c + 1) * 512], in_=sq[:])
                    bc = pbig.tile([128, 2, 512], F32, tag="mm2")
                    nc.tensor.matmul(bc[0:64, 0, :], ones_sb[0:1, 0:64],
                                     qn_inv[0:1, 2 * t, c * 512:(c + 1) * 512],
                                     start=True, stop=True)
                    nc.tensor.matmul(bc[64:128, 0, :], ones_sb[0:1, 0:64],
                                     qn_inv[0:1, 2 * t + 1, c * 512:(c + 1) * 512],
                                     start=True, stop=True, skip_group_check=True)
                    nc.vector.tensor_mul(out=qhat[:, t, c * 512:(c + 1) * 512],
                                         in0=bc[:, 0, :],
                                         in1=qp[:, t, c * 512:(c + 1) * 512])

                # ---- attention blocks: scores into 2-bank psum pairs, one
                # broadcast-view DVE op scales+narrows 1024 cols at a time.
                # accum of block b-1 is interleaved into scores of block b at
                # pair granularity so 16 attn buffers suffice (the slot a
                # scale is about to overwrite was read one PE step earlier).
                STEPS = [(j2, jj) for j2 in range(NKT // 2) for jj in range(2)]

                def emit_score_pair(p, c, j2, jj, si):
                    h = 2 * p + jj
                    sp = pbig.tile([128, 2, 512], F32, tag="mm2")
                    for u in range(2):
                        j = 2 * j2 + u
                        nc.tensor.matmul(
                            sp[:, u, :],
                            kp[jj * 64:(jj + 1) * 64, p, j * 128:(j + 1) * 128],
                            qhat[jj * 64:(jj + 1) * 64, p, c * 512:(c + 1) * 512],
                            start=True, stop=True, skip_group_check=(u == 1))
                    at = patt.tile([128, 2, 512], BF16, tag="attn")
                    if si % 8 < 5:
                        # one merged DVE op: per-k-row scale via broadcast view
                        nc.vector.tensor_mul(
                            out=at[:], in0=sp[:],
                            in1=kn_inv[:, 2 * j2:2 * j2 + 2, h:h + 1].to_broadcast(
                                [128, 2, 512]))
                    else:
                        for u in range(2):
                            nc.scalar.mul(out=at[:, u, :], in_=sp[:, u, :],
                                          mul=kn_inv[:, 2 * j2 + u, h:h + 1])
                    return at

                def emit_accum_pair(p, ops, at, j2, jj, si):
                    h = 2 * p + jj
                    for u in range(2):
                        nc.tensor.matmul(
                            ops[jj * 64:(jj + 1) * 64, :],
                            vp[:, 2 * j2 + u, h * 64:(h + 1) * 64],
                            at[:, u, :],
                            start=(si == 0 and u == 0) if jj == 0 else
                                  (si == 1 and u == 0),
                            stop=(si == len(STEPS) - 2 and u == 1) if jj == 0 else
                                 (si == len(STEPS) - 1 and u == 1),
                            skip_group_check=(jj == 1))

                # ---- emission schedule ----
                for g in range(4):
                    ln_group(xk, g, zkT)
                for g in range(2):
                    ln_group(xq, g, zqT)
                for ch in range(KC):
                    sq2 = proj_chunk(zkr, ch, kp)
                    knorm_chunk(ch, sq2)
                knorm_finish()
                for c in range(QC):
                    sq2 = proj_chunk(zqr, c, qp)
                    for t in range(2):
                        qnorm_qhat(t, c, sq2)
                for g in range(4):
                    ln_group(xv, g, zvT)

                blocks = [(p, c) for p in range(2) for c in range(QC)]

                # block 0 scores stand alone; v-projection slots in while its
                # scales drain, then blocks pipeline at pair granularity.
                p0, c0 = blocks[0]
                prev_tiles = [emit_score_pair(p0, c0, j2, jj, si)
                              for si, (j2, jj) in enumerate(STEPS)]
                prev_p, prev_c = p0, c0
                vproj()
                for (p, c) in blocks[1:]:
                    ops = pacc.tile([128, 512], F32, tag="acc")
                    cur = []
                    for si, (j2, jj) in enumerate(STEPS):
                        cur.append(emit_score_pair(p, c, j2, jj, si))
                        emit_accum_pair(prev_p, ops, prev_tiles[si], j2, jj, si)
                    nc.scalar.copy(out=outT[:, prev_p, prev_c * 512:(prev_c + 1) * 512],
                                   in_=ops[:])
                    prev_tiles, prev_p, prev_c = cur, p, c
                def final_pair(m2):
                    # two row-tiles of the output projection share a psum pair
                    fp = pbig.tile([128, 2, 512], F32, tag="mm2")
                    for u in range(2):
                        m = 2 * m2 + u
                        for t in range(2):
                            nc.tensor.matmul(fp[:, u, :],
                                             outT[:, t, m * 128:(m + 1) * 128],
                                             wout_sb[:, t, :], start=(t == 0),
                                             stop=(t == 1), skip_group_check=(u == 1))
                    o_sb = pfin.tile([128, 2, DIM], F32, tag="o")
                    nc.scalar.copy(out=o_sb[:], in_=fp[:])
                    dst = out_d[m2 * 256:(m2 + 1) * 256, :].rearrange(
                        "(a p) d -> p a d", p=128)
                    nc.gpsimd.dma_start(out=dst, in_=o_sb[:])

                # q-chunk 0 of outT is complete once block (1,0)'s accum landed
                # (copied inside the last loop iteration) -> its final
                # projection overlaps the last block's scores/accum.
                final_pair(0)
                final_pair(1)
                ops = pacc.tile([128, 512], F32, tag="acc")
                for si, (j2, jj) in enumerate(STEPS):
                    emit_accum_pair(prev_p, ops, prev_tiles[si], j2, jj, si)
                nc.scalar.copy(out=outT[:, prev_p, prev_c * 512:(prev_c + 1) * 512],
                               in_=ops[:])
                final_pair(2)
                final_pair(3)

            n_iter = reps if loop_reps is None else loop_reps
            if n_iter == 0:      # straight-line build (offline sim only)
                emit_body()
            else:
                with tc.For_i(0, n_iter, 1) as _i:
                    emit_body()

    nc.compile()
    return nc


def _get_nc(reps: int = 1, loop_reps=None, use_bias: bool = False):
    key = (reps, loop_reps, use_bias)
    if key not in _CACHE:
        _CACHE[key] = _build(reps, loop_reps, use_bias)
    return _CACHE[key]


def _host_prep(q, k, v, ln_gamma, ln_beta, W_qkv, W_out):
    q = np.asarray(q, np.float32)
    k = np.asarray(k, np.float32)
    v = np.asarray(v, np.float32)
    g = np.asarray(ln_gamma, np.float32)
    bt = np.asarray(ln_beta, np.float32)
    Wq = np.asarray(W_qkv, np.float32)[:, :HEADS * DH]
    Wo = np.asarray(W_out, np.float32)

    sel1 = np.zeros((128, 2), np.float32)
    sel1[0:64, 0] = 1.0
    sel1[64:128, 1] = 1.0
    selk = np.zeros((128, 2, LH), np.float32)
    for t in range(2):
        for j in range(2):
            selk[j * 64:(j + 1) * 64, t, 2 * t + j] = 1.0
    ones = np.ones((1, DIM), np.float32)

    bf = ml_dtypes.bfloat16
    in_maps = []
    for core in range(N_CORES):
        b, grp = core // HG, core % HG
        csl = slice(grp * IS, (grp + 1) * IS)
        Wq_g = Wq[:, csl]
        wqe = np.ascontiguousarray(
            (g[:, None] * Wq_g).reshape(4, 128, IS).transpose(1, 0, 2)).astype(bf)
        wb = (bt @ Wq_g).reshape(1, IS).astype(bf)
        wo = np.ascontiguousarray(
            Wo[csl, :].reshape(2, 128, DIM).transpose(1, 0, 2)).astype(bf)
        in_maps.append({
            "xq": np.ascontiguousarray(q[b]),
            "xk": np.ascontiguousarray(k[b]),
            "xv": np.ascontiguousarray(v[b]),
            "wqe": wqe,
            "wout": wo,
            "wbv": wb,
            "sel1": sel1.astype(bf),
            "selk": selk.astype(bf),
            "ones": ones.astype(bf),
        })
    return in_maps


def kernel(q, k, v, ln_gamma, ln_beta, W_qkv, W_out, b_out):
    in_maps = _host_prep(q, k, v, ln_gamma, ln_beta, W_qkv, W_out)
    use_bias = any(np.any(m["wbv"]) for m in in_maps)
    nc = _get_nc(1, use_bias=use_bias)
    res = bass_utils.run_bass_kernel_spmd(nc, in_maps, core_ids=list(range(N_CORES)))
    b_out = np.asarray(b_out, np.float32)
    out = np.empty((B, NQ, DIM), np.float32)
    for b in range(B):
        out[b] = res.results[b * HG]["out"] + res.results[b * HG + 1]["out"] + b_out
    return out
